# revision 9
# baseline (speedup 1.0000x reference)
"""Sequence-parallel sliding-window MHA kernel for Trainium2 (8 NeuronCores).

Problem (hardcoded): B=1, S=4096, D=1024, H=16, d_k=64, window=256 (left 127,
right 128), fp32 in/out.

Sharding: sequence-parallel. Core i owns queries [512*i, 512*(i+1)) and reads a
zero-padded, chunk-aligned key/value halo window of 768 tokens starting at
512*i - 128.  No collectives: each core writes its own 512-row output stripe.

v3: everything on-chip is fp16 (operands) with fp32 PSUM accumulation — PE is
full-rate at fp16, DMA bytes halve, DVE tensor ops get 16-bit fast modes, and
all four weight matrices fit in SBUF (one strided DMA each).

Device dataflow:
  QT[f,t]  = wqT.T @ qT        (lhsT = wq tile [c,f], rhs = qT [c,t])
  KT[f,s]  = wkT.T @ kT
  V [s,f]  = vT.T  @ wvT       (lhsT = vT tile [c,s], rhs = wvT [c,f]),
             stored with a ones-column every 65th slot (per head) so the
             attention AV matmul also produces the softmax denominator.
  per head h: ST[s,t] = KT_h.T @ QT_h (6 key chunks), exp(0.125*ST) on ACT,
             band-mask multiply on DVE, XT'[0:65,t] += V_aug.T @ expST
             (row 64 = denominator); normalize via DVE reciprocal + K=1
             PE broadcast matmul; odd heads DMA-moved to partitions 64:128.
  O[t,f]   = XT.T @ woT + b_o  (bias via K=1 ones matmul accumulation)
"""

import numpy as np

S = 4096
D = 1024
H = 16
DK = 64
NCORES = 8
T = 512          # queries per core
SK = 768         # padded key window per core (6 chunks of 128)
KS_OFF = -128    # key window start relative to query stripe start

_CACHE = {}


def _build_nc():
    import concourse.bass as bass
    import concourse.mybir as mybir
    import concourse.tile as tile
    from concourse import bacc

    f32 = mybir.dt.float32
    f16 = mybir.dt.float16

    nc = bacc.Bacc("TRN2", target_bir_lowering=False, debug=False)

    qT = nc.dram_tensor("qT", [D, T], f16, kind="ExternalInput")
    kT = nc.dram_tensor("kT", [D, SK], f16, kind="ExternalInput")
    vT = nc.dram_tensor("vT", [D, SK], f16, kind="ExternalInput")
    wqT = nc.dram_tensor("wqT", [D, D], f16, kind="ExternalInput")
    wkT = nc.dram_tensor("wkT", [D, D], f16, kind="ExternalInput")
    wvT = nc.dram_tensor("wvT", [D, D], f16, kind="ExternalInput")
    woT = nc.dram_tensor("woT", [D, D], f16, kind="ExternalInput")
    bq = nc.dram_tensor("bq", [D], f32, kind="ExternalInput")
    bk = nc.dram_tensor("bk", [D], f32, kind="ExternalInput")
    bv = nc.dram_tensor("bv", [D], f16, kind="ExternalInput")
    bo = nc.dram_tensor("bo", [D], f16, kind="ExternalInput")
    masks = nc.dram_tensor("masks", [6, 128, T], f16, kind="ExternalInput")
    out = nc.dram_tensor("out", [T, D], f32, kind="ExternalOutput")

    with tile.TileContext(nc) as tc:
        with (
            nc.allow_low_precision(reason="fp16 operands, fp32 accumulation"),
            tc.tile_pool(name="const", bufs=1) as cpool,
            tc.tile_pool(name="cache", bufs=1) as cache,
            tc.tile_pool(name="work", bufs=2) as work,
            tc.tile_pool(name="expp", bufs=3) as expp,
            tc.tile_pool(name="psum", bufs=2, space="PSUM") as psum,
            tc.tile_pool(name="psbc", bufs=1, space="PSUM") as psbc,
            tc.tile_pool(name="pspp", bufs=3, space="PSUM") as pspp,
        ):
            # ---- constants ----
            ones_sb = cpool.tile([128, 128], f16, tag="ones")
            nc.vector.memset(ones_sb[:], 1.0)
            bq_sb = cpool.tile([128, 8], f32, tag="bq")
            nc.scalar.dma_start(bq_sb[:], bq.rearrange("(o p) -> p o", p=128))
            bk_sb = cpool.tile([128, 8], f32, tag="bk")
            nc.scalar.dma_start(bk_sb[:], bk.rearrange("(o p) -> p o", p=128))
            bo_sb = cpool.tile([1, D], f16, tag="bo")
            nc.scalar.dma_start(bo_sb[:], bo[None, :])
            bv_sb = cpool.tile([1, D], f16, tag="bv")
            nc.scalar.dma_start(bv_sb[:], bv[None, :])
            masks_sb = cpool.tile([128, 6, T], f16, tag="masks")
            nc.scalar.dma_start(masks_sb[:], masks.rearrange("c p t -> p c t"))

            # ---- cached inputs / weights (single strided DMA each) ----
            qT_sb = cache.tile([128, 8, T], f16, tag="qT")
            nc.gpsimd.dma_start(qT_sb[:], qT.rearrange("(o p) t -> p o t", p=128))
            kT_sb = cache.tile([128, 8, SK], f16, tag="kT")
            nc.gpsimd.dma_start(kT_sb[:], kT.rearrange("(o p) t -> p o t", p=128))
            vT_sb = cache.tile([128, 8, SK], f16, tag="vT")
            nc.gpsimd.dma_start(vT_sb[:], vT.rearrange("(o p) t -> p o t", p=128))
            wv_sb = cache.tile([128, 8, D], f16, tag="wv")
            nc.sync.dma_start(wv_sb[:], wvT.rearrange("(o p) f -> p o f", p=128))
            wk_sb = cache.tile([128, 8, D], f16, tag="wk")
            nc.sync.dma_start(wk_sb[:], wkT.rearrange("(o p) f -> p o f", p=128))
            wq_sb = cache.tile([128, 8, D], f16, tag="wq")
            nc.sync.dma_start(wq_sb[:], wqT.rearrange("(o p) f -> p o f", p=128))
            wo_sb = cache.tile([128, 8, D], f16, tag="wo")
            nc.sync.dma_start(wo_sb[:], woT.rearrange("(o p) f -> p o f", p=128))

            # bv broadcast to all partitions via K=1 matmul
            bv_bc = cpool.tile([128, D], f32, tag="bv_bc")
            for half in range(2):
                ps = pspp.tile([128, 512], f32, tag="pp")
                nc.tensor.matmul(
                    ps[:], ones_sb[0:1, 0:128],
                    bv_sb[0:1, half * 512:(half + 1) * 512],
                    start=True, stop=True,
                )
                nc.scalar.copy(bv_bc[:, half * 512:(half + 1) * 512], ps[:])

            # ---- V projection:  V_sb[s, 65h:65h+64] = (v @ w_v.T)_h, 65h+64 = 1
            V_sb = cache.tile([128, 6, H * 65], f16, tag="V")
            ones_cols = V_sb.rearrange("p c (h e) -> p c h e", e=65)[:, :, :, 64:65]
            nc.vector.memset(ones_cols, 1.0)
            for fc in range(2):
                for s in range(6):
                    ps = pspp.tile([128, 512], f32, tag="pp")
                    for c in range(8):
                        nc.tensor.matmul(
                            ps[:], vT_sb[:, c, s * 128:(s + 1) * 128],
                            wv_sb[:, c, fc * 512:(fc + 1) * 512],
                            start=(c == 0), stop=(c == 7),
                        )
                    dst = V_sb.rearrange("p c (h e) -> p c h e", e=65)[
                        :, s, fc * 8:(fc + 1) * 8, 0:64]
                    srcp = ps.rearrange("p (h e) -> p h e", e=64)
                    bvb = bv_bc.rearrange("p (h e) -> p h e", e=64)[
                        :, fc * 8:(fc + 1) * 8, :]
                    nc.vector.tensor_tensor(dst, srcp, bvb, mybir.AluOpType.add)

            # ---- K projection: KT_sb[f, s] ----
            KT_sb = cache.tile([128, 8, SK], f16, tag="KT")
            for f in range(8):
                for s0, sw in ((0, 512), (512, 256)):
                    ps = pspp.tile([128, 512], f32, tag="pp")
                    for c in range(8):
                        nc.tensor.matmul(
                            ps[:, 0:sw], wk_sb[:, c, f * 128:(f + 1) * 128],
                            kT_sb[:, c, s0:s0 + sw],
                            start=(c == 0), stop=(c == 7),
                        )
                    nc.vector.tensor_scalar_add(
                        KT_sb[:, f, s0:s0 + sw], ps[:, 0:sw], bk_sb[:, f:f + 1])

            # ---- Q projection: QT_sb[f, t] ----
            QT_sb = cache.tile([128, 8, T], f16, tag="QT")
            for f in range(8):
                ps = pspp.tile([128, 512], f32, tag="pp")
                for c in range(8):
                    nc.tensor.matmul(
                        ps[:], wq_sb[:, c, f * 128:(f + 1) * 128],
                        qT_sb[:, c, :],
                        start=(c == 0), stop=(c == 7),
                    )
                nc.vector.tensor_scalar_add(
                    QT_sb[:, f, :], ps[:], bq_sb[:, f:f + 1])

            # ---- attention per head ----
            XT_sb = cache.tile([128, 8, T], f16, tag="qT")  # reuse qT slot
            for h in range(H):
                ft, r0 = h // 2, 64 * (h % 2)
                xt = psum.tile([128, 512], f32, tag="xt")
                for c6 in range(6):
                    st = psum.tile([128, 512], f32, tag="st")
                    nc.tensor.matmul(
                        st[:],
                        KT_sb[r0:r0 + 64, ft, c6 * 128:(c6 + 1) * 128],
                        QT_sb[r0:r0 + 64, ft, :],
                        start=True, stop=True,
                    )
                    exr = expp.tile([128, T], f16, tag="exr")
                    nc.scalar.activation(
                        exr[:], st[:], mybir.ActivationFunctionType.Exp,
                        scale=0.125)
                    ex = expp.tile([128, T], f16, tag="ex")
                    nc.vector.tensor_mul(ex[:], exr[:], masks_sb[:, c6, :])
                    nc.tensor.matmul(
                        xt[0:65, :],
                        V_sb[:, c6, 65 * h:65 * h + 65],
                        ex[:],
                        start=(c6 == 0), stop=(c6 == 5),
                    )
                dn = work.tile([128, 512], f16, tag="dn")
                nc.vector.reciprocal(dn[64:65, :], xt[64:65, :])
                bc_ps = psbc.tile([128, 512], f32, tag="bc")
                nc.tensor.matmul(
                    bc_ps[0:64, :], ones_sb[64:65, 0:64], dn[64:65, :],
                    start=True, stop=True,
                )
                bc = work.tile([128, 512], f32, tag="bc_sb")
                nc.scalar.copy(bc[0:64, :], bc_ps[0:64, :])
                if h % 2 == 0:
                    nc.vector.tensor_mul(
                        XT_sb[0:64, ft, :], xt[0:64, :], bc[0:64, :])
                else:
                    xm = work.tile([64, 512], f16, tag="xtmp")
                    nc.vector.tensor_mul(xm[:], xt[0:64, :], bc[0:64, :])
                    nc.gpsimd.dma_start(XT_sb[64:128, ft, :], xm[:])

            # ---- output projection: O[t, f] ----
            for fc in range(2):
                for tt in range(4):
                    ps = pspp.tile([128, 512], f32, tag="pp")
                    for j in range(8):
                        nc.tensor.matmul(
                            ps[:],
                            XT_sb[:, j, tt * 128:(tt + 1) * 128],
                            wo_sb[:, j, fc * 512:(fc + 1) * 512],
                            start=(j == 0), stop=False,
                        )
                    nc.tensor.matmul(
                        ps[:], ones_sb[0:1, 0:128],
                        bo_sb[0:1, fc * 512:(fc + 1) * 512],
                        start=False, stop=True,
                    )
                    o_sb = work.tile([128, 512], f32, tag="o_sb")
                    nc.scalar.copy(o_sb[:], ps[:])
                    nc.sync.dma_start(
                        out[tt * 128:(tt + 1) * 128, fc * 512:(fc + 1) * 512],
                        o_sb[:])

    nc.compile()
    return nc


def _make_in_maps(q, k, v, w_q, b_q, w_k, b_k, w_v, b_v, w_o, b_o):
    qT = np.ascontiguousarray(q[0].T).astype(np.float16)   # [D, S]
    kT = np.ascontiguousarray(k[0].T).astype(np.float16)
    vT = np.ascontiguousarray(v[0].T).astype(np.float16)
    wqT = np.ascontiguousarray(w_q.T).astype(np.float16)   # [c_in, f_out]
    wkT = np.ascontiguousarray(w_k.T).astype(np.float16)
    wvT = np.ascontiguousarray(w_v.T).astype(np.float16)
    woT = np.ascontiguousarray(w_o.T).astype(np.float16)

    in_maps = []
    for core in range(NCORES):
        t0 = core * T
        ks0 = t0 + KS_OFF
        kTi = np.zeros((D, SK), np.float16)
        vTi = np.zeros((D, SK), np.float16)
        lo, hi = max(0, ks0), min(S, ks0 + SK)
        kTi[:, lo - ks0:hi - ks0] = kT[:, lo:hi]
        vTi[:, lo - ks0:hi - ks0] = vT[:, lo:hi]

        m = np.zeros((6, 128, T), np.float16)
        s_glob = ks0 + np.arange(SK).reshape(6, 128, 1)
        t_glob = t0 + np.arange(T).reshape(1, 1, T)
        d = s_glob - t_glob
        m[:] = ((d >= -127) & (d <= 128) & (s_glob >= 0) & (s_glob < S))

        in_maps.append({
            "qT": np.ascontiguousarray(qT[:, t0:t0 + T]),
            "kT": kTi, "vT": vTi,
            "wqT": wqT, "wkT": wkT, "wvT": wvT, "woT": woT,
            "bq": np.asarray(b_q, np.float32), "bk": np.asarray(b_k, np.float32),
            "bv": np.asarray(b_v, np.float16), "bo": np.asarray(b_o, np.float16),
            "masks": m,
        })
    return in_maps


def kernel(q, k, v, w_q, b_q, w_k, b_k, w_v, b_v, w_o, b_o, **trace_kw):
    from concourse.bass_utils import run_bass_kernel_spmd

    if "nc" not in _CACHE:
        _CACHE["nc"] = _build_nc()
    nc = _CACHE["nc"]

    in_maps = _make_in_maps(q, k, v, w_q, b_q, w_k, b_k, w_v, b_v, w_o, b_o)
    res = run_bass_kernel_spmd(nc, in_maps, list(range(NCORES)), **trace_kw)
    _CACHE["last_result"] = res
    stripes = [res.results[i]["out"] for i in range(NCORES)]
    return np.concatenate(stripes, axis=0)[None].astype(np.float32)


# revision 18
# speedup vs baseline: 670.6376x; 670.6376x over previous
"""Sequence-parallel sliding-window MHA kernel for Trainium2 (8 NeuronCores).

Problem (hardcoded): B=1, S=4096, D=1024, H=16, d_k=64, window=256 (left 127,
right 128), fp32 in/out.

Sharding: sequence-parallel. Core i owns queries [512*i, 512*(i+1)) and reads a
zero-padded, chunk-aligned key/value halo window of 768 tokens starting at
512*i - 128.  No collectives: each core writes its own 512-row output stripe.

v3: everything on-chip is fp16 (operands) with fp32 PSUM accumulation — PE is
full-rate at fp16, DMA bytes halve, DVE tensor ops get 16-bit fast modes, and
all four weight matrices fit in SBUF (one strided DMA each).

Device dataflow:
  QT[f,t]  = wqT.T @ qT        (lhsT = wq tile [c,f], rhs = qT [c,t])
  KT[f,s]  = wkT.T @ kT
  V [s,f]  = vT.T  @ wvT       (lhsT = vT tile [c,s], rhs = wvT [c,f]),
             stored with a ones-column every 65th slot (per head) so the
             attention AV matmul also produces the softmax denominator.
  per head h: ST[s,t] = KT_h.T @ QT_h (6 key chunks), exp(0.125*ST) on ACT,
             band-mask multiply on DVE, XT'[0:65,t] += V_aug.T @ expST
             (row 64 = softmax denominator); normalize: DVE reciprocal,
             DMA hop of the denom row to partition 0, GpSimd
             partition_broadcast, DVE multiply; odd heads DMA-moved to
             partitions 64:128 of XT.
  O[t,f]   = XT.T @ woT + b_o  (bias via K=1 ones matmul accumulation)
"""

import numpy as np

S = 4096
D = 1024
H = 16
DK = 64
NCORES = 8
T = 512          # queries per core
SK = 768         # padded key window per core (6 chunks of 128)
KS_OFF = -128    # key window start relative to query stripe start

_CACHE = {}


def _build_nc():
    import concourse.bass as bass
    import concourse.mybir as mybir
    import concourse.tile as tile
    from concourse import bacc

    f32 = mybir.dt.float32
    f16 = mybir.dt.float16

    nc = bacc.Bacc("TRN2", target_bir_lowering=False, debug=False)

    qT = nc.dram_tensor("qT", [D, T], f16, kind="ExternalInput")
    kT = nc.dram_tensor("kT", [D, SK], f16, kind="ExternalInput")
    vT = nc.dram_tensor("vT", [D, SK], f16, kind="ExternalInput")
    wqT = nc.dram_tensor("wqT", [D, D], f16, kind="ExternalInput")
    wkT = nc.dram_tensor("wkT", [D, D], f16, kind="ExternalInput")
    wvT = nc.dram_tensor("wvT", [D, D], f16, kind="ExternalInput")
    woT = nc.dram_tensor("woT", [D, D], f16, kind="ExternalInput")
    bq = nc.dram_tensor("bq", [D], f32, kind="ExternalInput")
    bk = nc.dram_tensor("bk", [D], f32, kind="ExternalInput")
    bv = nc.dram_tensor("bv", [D], f16, kind="ExternalInput")
    bo = nc.dram_tensor("bo", [D], f16, kind="ExternalInput")
    masks = nc.dram_tensor("masks", [6, 128, T], f16, kind="ExternalInput")
    out = nc.dram_tensor("out", [T, D], f32, kind="ExternalOutput")

    with tile.TileContext(nc) as tc:
        with (
            nc.allow_low_precision(reason="fp16 operands, fp32 accumulation"),
            tc.tile_pool(name="const", bufs=1) as cpool,
            tc.tile_pool(name="cache", bufs=1) as cache,
            tc.tile_pool(name="work", bufs=2) as work,
            tc.tile_pool(name="expp", bufs=6) as expp,
            tc.tile_pool(name="psum", bufs=2, space="PSUM") as psum,
            tc.tile_pool(name="psst", bufs=3, space="PSUM") as psst,
            tc.tile_pool(name="pspp", bufs=3, space="PSUM") as pspp,
        ):
            # ---- constants ----
            ones_sb = cpool.tile([128, 128], f16, tag="ones")
            nc.vector.memset(ones_sb[:], 1.0)
            bq_sb = cpool.tile([128, 8], f32, tag="bq")
            nc.scalar.dma_start(bq_sb[:], bq.rearrange("(o p) -> p o", p=128))
            bk_sb = cpool.tile([128, 8], f32, tag="bk")
            nc.scalar.dma_start(bk_sb[:], bk.rearrange("(o p) -> p o", p=128))
            bo_sb = cpool.tile([1, D], f16, tag="bo")
            nc.scalar.dma_start(bo_sb[:], bo[None, :])
            bv_sb = cpool.tile([1, D], f16, tag="bv")
            nc.scalar.dma_start(bv_sb[:], bv[None, :])
            masks_sb = cpool.tile([128, 6, T], f16, tag="masks")
            nc.scalar.dma_start(masks_sb[:], masks.rearrange("c p t -> p c t"))

            # ---- cached inputs / weights (single strided DMA each) ----
            vT_sb = cache.tile([128, 8, SK], f16, tag="vT")
            nc.gpsimd.dma_start(vT_sb[:], vT.rearrange("(o p) t -> p o t", p=128))
            kT_sb = cache.tile([128, 8, SK], f16, tag="kT")
            nc.gpsimd.dma_start(kT_sb[:], kT.rearrange("(o p) t -> p o t", p=128))
            qT_sb = cache.tile([128, 8, T], f16, tag="qT")
            nc.gpsimd.dma_start(qT_sb[:], qT.rearrange("(o p) t -> p o t", p=128))
            wv_sb = cache.tile([128, 8, D], f16, tag="wv")
            nc.sync.dma_start(wv_sb[:], wvT.rearrange("(o p) f -> p o f", p=128))
            wk_sb = cache.tile([128, 8, D], f16, tag="wk")
            nc.sync.dma_start(wk_sb[:], wkT.rearrange("(o p) f -> p o f", p=128))
            wq_sb = cache.tile([128, 8, D], f16, tag="wq")
            nc.sync.dma_start(wq_sb[:], wqT.rearrange("(o p) f -> p o f", p=128))
            wo_sb = cache.tile([128, 8, D], f16, tag="wo")
            nc.sync.dma_start(wo_sb[:], woT.rearrange("(o p) f -> p o f", p=128))

            # bv broadcast to all partitions via K=1 matmul
            bv_bc = cpool.tile([128, D], f32, tag="bv_bc")
            for half in range(2):
                ps = pspp.tile([128, 512], f32, tag="pp")
                nc.tensor.matmul(
                    ps[:], ones_sb[0:1, 0:128],
                    bv_sb[0:1, half * 512:(half + 1) * 512],
                    start=True, stop=True,
                )
                nc.scalar.copy(bv_bc[:, half * 512:(half + 1) * 512], ps[:])

            # ---- V projection:  V_sb[s, 65h:65h+64] = (v @ w_v.T)_h, 65h+64 = 1
            V_sb = cache.tile([128, 6, H * 65], f16, tag="V")
            ones_cols = V_sb.rearrange("p c (h e) -> p c h e", e=65)[:, :, :, 64:65]
            nc.vector.memset(ones_cols, 1.0)
            for fc in range(2):
                for s in range(6):
                    ps = pspp.tile([128, 512], f32, tag="pp")
                    for c in range(8):
                        nc.tensor.matmul(
                            ps[:], vT_sb[:, c, s * 128:(s + 1) * 128],
                            wv_sb[:, c, fc * 512:(fc + 1) * 512],
                            start=(c == 0), stop=(c == 7),
                        )
                    dst = V_sb.rearrange("p c (h e) -> p c h e", e=65)[
                        :, s, fc * 8:(fc + 1) * 8, 0:64]
                    srcp = ps.rearrange("p (h e) -> p h e", e=64)
                    bvb = bv_bc.rearrange("p (h e) -> p h e", e=64)[
                        :, fc * 8:(fc + 1) * 8, :]
                    nc.vector.tensor_tensor(dst, srcp, bvb, mybir.AluOpType.add)

            # ---- K projection: KT_sb[f, s] ----
            KT_sb = cache.tile([128, 8, SK], f16, tag="KT")
            for f in range(8):
                for s0, sw in ((0, 512), (512, 256)):
                    ps = pspp.tile([128, 512], f32, tag="pp")
                    for c in range(8):
                        nc.tensor.matmul(
                            ps[:, 0:sw], wk_sb[:, c, f * 128:(f + 1) * 128],
                            kT_sb[:, c, s0:s0 + sw],
                            start=(c == 0), stop=(c == 7),
                        )
                    nc.vector.tensor_scalar_add(
                        KT_sb[:, f, s0:s0 + sw], ps[:, 0:sw], bk_sb[:, f:f + 1])

            # ---- Q projection: QT_sb[f, t] ----
            QT_sb = cache.tile([128, 8, T], f16, tag="QT")
            for f in range(8):
                ps = pspp.tile([128, 512], f32, tag="pp")
                for c in range(8):
                    nc.tensor.matmul(
                        ps[:], wq_sb[:, c, f * 128:(f + 1) * 128],
                        qT_sb[:, c, :],
                        start=(c == 0), stop=(c == 7),
                    )
                nc.vector.tensor_scalar_add(
                    QT_sb[:, f, :], ps[:], bq_sb[:, f:f + 1])

            # ---- attention per head ----
            XT_sb = cache.tile([128, 8, T], f16, tag="qT")  # reuse qT slot
            head_order = [p * 2 + o for p in range(8) for o in (1, 0)]
            for h in head_order:
                ft, r0 = h // 2, 64 * (h % 2)
                xt = psum.tile([128, 512], f32, tag="xt", name=f"xt{h}")
                for c6 in range(6):
                    st = psst.tile([128, 512], f32, tag="st", name=f"st{h}_{c6}")
                    nc.tensor.matmul(
                        st[:],
                        KT_sb[r0:r0 + 64, ft, c6 * 128:(c6 + 1) * 128],
                        QT_sb[r0:r0 + 64, ft, :],
                        start=True, stop=True,
                    )
                    exr = expp.tile([128, T], f16, tag="exr", name=f"exr{h}_{c6}")
                    nc.scalar.activation(
                        exr[:], st[:], mybir.ActivationFunctionType.Exp,
                        scale=0.125)
                    ex = expp.tile([128, T], f16, tag="ex", name=f"ex{h}_{c6}")
                    nc.vector.tensor_mul(ex[:], exr[:], masks_sb[:, c6, :])
                    nc.tensor.matmul(
                        xt[0:65, :],
                        V_sb[:, c6, 65 * h:65 * h + 65],
                        ex[:],
                        start=(c6 == 0), stop=(c6 == 5),
                    )
                dn = work.tile([128, 512], f32, tag="dn", name=f"dn{h}")
                nc.vector.reciprocal(dn[64:65, :], xt[64:65, :])
                dnr = work.tile([1, 512], f32, tag="dnr", name=f"dnr{h}")
                nc.sync.dma_start(dnr[0:1, :], dn[64:65, :])
                bc = work.tile([128, 512], f32, tag="bc_sb", name=f"bc{h}")
                nc.gpsimd.partition_broadcast(bc[0:64, :], dnr[0:1, :])
                if h % 2 == 0:
                    nc.vector.tensor_mul(
                        XT_sb[0:64, ft, :], xt[0:64, :], bc[0:64, :])
                else:
                    xm = work.tile([64, 512], f16, tag="xtmp", name=f"xm{h}")
                    nc.vector.tensor_mul(xm[:], xt[0:64, :], bc[0:64, :])
                    nc.gpsimd.dma_start(XT_sb[64:128, ft, :], xm[:])

            # ---- output projection: O[t, f] ----
            for fc in range(2):
                for tt in range(4):
                    ps = pspp.tile([128, 512], f32, tag="pp")
                    for j in range(8):
                        nc.tensor.matmul(
                            ps[:],
                            XT_sb[:, j, tt * 128:(tt + 1) * 128],
                            wo_sb[:, j, fc * 512:(fc + 1) * 512],
                            start=(j == 0), stop=False,
                        )
                    nc.tensor.matmul(
                        ps[:], ones_sb[0:1, 0:128],
                        bo_sb[0:1, fc * 512:(fc + 1) * 512],
                        start=False, stop=True,
                    )
                    o_sb = work.tile([128, 512], f32, tag="o_sb")
                    nc.scalar.copy(o_sb[:], ps[:])
                    nc.sync.dma_start(
                        out[tt * 128:(tt + 1) * 128, fc * 512:(fc + 1) * 512],
                        o_sb[:])

    nc.compile()
    return nc


def _make_in_maps(q, k, v, w_q, b_q, w_k, b_k, w_v, b_v, w_o, b_o):
    qT = np.ascontiguousarray(q[0].T).astype(np.float16)   # [D, S]
    kT = np.ascontiguousarray(k[0].T).astype(np.float16)
    vT = np.ascontiguousarray(v[0].T).astype(np.float16)
    wqT = np.ascontiguousarray(w_q.T).astype(np.float16)   # [c_in, f_out]
    wkT = np.ascontiguousarray(w_k.T).astype(np.float16)
    wvT = np.ascontiguousarray(w_v.T).astype(np.float16)
    woT = np.ascontiguousarray(w_o.T).astype(np.float16)

    in_maps = []
    for core in range(NCORES):
        t0 = core * T
        ks0 = t0 + KS_OFF
        kTi = np.zeros((D, SK), np.float16)
        vTi = np.zeros((D, SK), np.float16)
        lo, hi = max(0, ks0), min(S, ks0 + SK)
        kTi[:, lo - ks0:hi - ks0] = kT[:, lo:hi]
        vTi[:, lo - ks0:hi - ks0] = vT[:, lo:hi]

        m = np.zeros((6, 128, T), np.float16)
        s_glob = ks0 + np.arange(SK).reshape(6, 128, 1)
        t_glob = t0 + np.arange(T).reshape(1, 1, T)
        d = s_glob - t_glob
        m[:] = ((d >= -127) & (d <= 128) & (s_glob >= 0) & (s_glob < S))

        in_maps.append({
            "qT": np.ascontiguousarray(qT[:, t0:t0 + T]),
            "kT": kTi, "vT": vTi,
            "wqT": wqT, "wkT": wkT, "wvT": wvT, "woT": woT,
            "bq": np.asarray(b_q, np.float32), "bk": np.asarray(b_k, np.float32),
            "bv": np.asarray(b_v, np.float16), "bo": np.asarray(b_o, np.float16),
            "masks": m,
        })
    return in_maps


def kernel(q, k, v, w_q, b_q, w_k, b_k, w_v, b_v, w_o, b_o, **trace_kw):
    from concourse.bass_utils import run_bass_kernel_spmd

    if "nc" not in _CACHE:
        _CACHE["nc"] = _build_nc()
    nc = _CACHE["nc"]

    in_maps = _make_in_maps(q, k, v, w_q, b_q, w_k, b_k, w_v, b_v, w_o, b_o)
    res = run_bass_kernel_spmd(nc, in_maps, list(range(NCORES)), **trace_kw)
    _CACHE["last_result"] = res
    stripes = [res.results[i]["out"] for i in range(NCORES)]
    return np.concatenate(stripes, axis=0)[None].astype(np.float32)
